# revision 6
# baseline (speedup 1.0000x reference)
"""BlockReLU (nn_BlockReLU_V1) Trainium2 Bass kernel.

Full input: activation [16, 128, 128, 128] f32 (N, C, H, W).
Per-channel block gating:
  ch   0- 31: 1x1 blocks  -> plain ReLU
  ch  32- 63: 2x2 blocks  -> zero block where block-sum < 0
  ch  64- 95: 4x4 blocks
  ch  96-111: 2x4 (h x w) blocks
  ch 112-127: identity passthrough

Sharding: pure data-parallel over batch N across 8 NeuronCores
(2 samples/core). Inside a core, each (sample, channel-group) is one
[128, fs] SBUF tile: partition = (channel, H-chunk) with
chunks-per-channel chosen so channels*chunks = 128; the free dim is
(rows-in-chunk, W). Chunk row counts are multiples of the block height,
so all pooling is partition-local:
  - block sums via pairwise add trees along rows then columns (strided
    tensor_tensor adds on DVE),
  - gating fused into one scalar_tensor_tensor per row-offset:
    out = (broadcast(sum) >= 0) * x, mask broadcast via 0-step AP dims.
DMA: each group tile is a contiguous HBM block -> plain [128, fs]
HWDGE transfers (2 MiB / 1 MiB); loads on the SP ring, stores on the
ACT ring.
"""

import sys

if "/opt/trn_rl_repo" not in sys.path:
    sys.path.insert(0, "/opt/trn_rl_repo")

import numpy as np

import concourse.bacc as bacc
import concourse.mybir as mybir
from concourse.tile import TileContext

N_CORES = 8
NS = 2          # samples per core
C, H, W = 128, 128, 128
F32 = mybir.dt.float32

# (channel_start, n_channels, block_h, block_w); block_h None => identity
GROUPS = [
    (0, 32, 1, 1),
    (32, 32, 2, 2),
    (64, 32, 4, 4),
    (96, 16, 2, 4),
    (112, 16, None, None),
]


def _emit_group(nc, pools, act, out, c0, gc, bh, bw):
    kc = 128 // gc          # H-chunks per channel
    r = H // kc             # rows per chunk (== gc here)
    fs = NS * r * W         # free elements per partition (both samples)

    px, ps1, ps2, pw1, pw2 = pools
    x = px.tile([128, fs], F32, tag="x")

    # per sample, the group block is contiguous in HBM: [gc, H, W] from
    # channel c0; both samples land in one tile: free dim = (n, rows, W)
    src = (
        act[:, c0 : c0 + gc]
        .rearrange("n c (k r) w -> n c k (r w)", k=kc)
        .transpose([1, 2, 0, 3])
        .rearrange("c k n f -> (c k) n f")
    )
    dst = (
        out[:, c0 : c0 + gc]
        .rearrange("n c (k r) w -> n c k (r w)", k=kc)
        .transpose([1, 2, 0, 3])
        .rearrange("c k n f -> (c k) n f")
    )

    nc.sync.dma_start(x[:].rearrange("p (n f) -> p n f", n=NS), src)

    if bh is not None and bh * bw > 1:
        nh = NS * r // bh   # h-blocks per partition across both samples
        nw = W // bw

        # H reduction: pairwise row adds until one row per h-block
        cur, rows = x, NS * r
        while rows > nh:
            nxt = (ps1 if rows == NS * r else ps2).tile(
                [128, (rows // 2) * W], F32, tag="s1" if rows == NS * r else "s2"
            )
            v = cur[:].rearrange("p (b t w) -> p b t w", t=2, w=W)
            nc.vector.tensor_add(
                nxt[:].rearrange("p (b w) -> p b w", w=W),
                v[:, :, 0, :],
                v[:, :, 1, :],
            )
            cur, rows = nxt, rows // 2

        # W reduction: pairwise column adds until one value per block
        cols = W
        while cols > nw:
            nxt = (pw1 if cols == W else pw2).tile(
                [128, nh * (cols // 2)], F32, tag="w1" if cols == W else "w2"
            )
            v = cur[:].rearrange("p (b c t) -> p b c t", b=nh, t=2)
            nc.vector.tensor_add(
                nxt[:].rearrange("p (b c) -> p b c", b=nh),
                v[:, :, :, 0],
                v[:, :, :, 1],
            )
            cur, cols = nxt, cols // 2

        # Gate: out = (block_sum >= 0) * x, one op per row offset in block
        msum = cur[:].rearrange("p (b wb) -> p b wb", wb=nw)
        mbc = msum.unsqueeze(3).broadcast_to([128, nh, nw, bw])
        for hi in range(bh):
            xv = (
                x[:]
                .rearrange("p (b t w) -> p b t w", t=bh, w=W)[:, :, hi, :]
                .rearrange("p b (wb wi) -> p b wb wi", wi=bw)
            )
            nc.vector.scalar_tensor_tensor(
                xv, mbc, 0.0, xv, mybir.AluOpType.is_ge, mybir.AluOpType.mult
            )
    elif bh == 1:
        # ReLU channels
        nc.vector.tensor_scalar_max(x[:], x[:], 0.0)
    # else identity: store as-is

    nc.scalar.dma_start(dst, x[:].rearrange("p (n f) -> p n f", n=NS))


def build_bass():
    nc = bacc.Bacc(
        "TRN2", target_bir_lowering=False, debug=False, num_devices=N_CORES
    )
    act = nc.dram_tensor("activation", [NS, C, H, W], F32, kind="ExternalInput")
    out = nc.dram_tensor("out", [NS, C, H, W], F32, kind="ExternalOutput")
    with TileContext(nc) as tc:
        with (
            tc.tile_pool(name="x", bufs=3) as px,
            tc.tile_pool(name="s1", bufs=2) as ps1,
            tc.tile_pool(name="s2", bufs=2) as ps2,
            tc.tile_pool(name="w1", bufs=2) as pw1,
            tc.tile_pool(name="w2", bufs=2) as pw2,
        ):
            pools = (px, ps1, ps2, pw1, pw2)
            for c0, gc, bh, bw in GROUPS:
                _emit_group(nc, pools, act, out, c0, gc, bh, bw)
    nc.compile()
    return nc


_NC = None


def _get_nc():
    global _NC
    if _NC is None:
        _NC = build_bass()
    return _NC


def run(activation, trace=False, **spmd_kwargs):
    from concourse.bass_utils import run_bass_kernel_spmd

    activation = np.ascontiguousarray(np.asarray(activation), dtype=np.float32)
    assert activation.shape == (N_CORES * NS, C, H, W), activation.shape
    nc = _get_nc()
    in_maps = [
        {"activation": activation[i * NS : (i + 1) * NS]} for i in range(N_CORES)
    ]
    res = run_bass_kernel_spmd(
        nc, in_maps, core_ids=list(range(N_CORES)), trace=trace, **spmd_kwargs
    )
    full = np.concatenate([r["out"] for r in res.results], axis=0)
    return full, res


def kernel(activation):
    return run(activation)[0]


if __name__ == "__main__":
    rng = np.random.default_rng(0)
    a = rng.standard_normal((16, 128, 128, 128), dtype=np.float32)
    y = kernel(a)
    print("ran:", y.shape, y.dtype)


# revision 10
# speedup vs baseline: 1.2482x; 1.2482x over previous
"""BlockReLU (nn_BlockReLU_V1) Trainium2 Bass kernel.

Full input: activation [16, 128, 128, 128] f32 (N, C, H, W).
Per-channel block gating:
  ch   0- 31: 1x1 blocks  -> plain ReLU
  ch  32- 63: 2x2 blocks  -> zero block where block-sum < 0
  ch  64- 95: 4x4 blocks
  ch  96-111: 2x4 (h x w) blocks
  ch 112-127: identity passthrough

Sharding: pure data-parallel over batch N across 8 NeuronCores
(2 samples/core). Inside a core, each (sample, channel-group) is one
[128, fs] SBUF tile: partition = (channel, H-chunk) with
chunks-per-channel chosen so channels*chunks = 128; the free dim is
(rows-in-chunk, W). Chunk row counts are multiples of the block height,
so all pooling is partition-local:
  - block sums via pairwise add trees along rows then columns (strided
    tensor_tensor adds on DVE),
  - gating fused into one scalar_tensor_tensor per row-offset:
    out = (broadcast(sum) >= 0) * x, mask broadcast via 0-step AP dims.
DMA: each group tile is a contiguous HBM block -> plain [128, fs]
HWDGE transfers (2 MiB / 1 MiB); loads on the SP ring, stores on the
ACT ring.
"""

import sys

if "/opt/trn_rl_repo" not in sys.path:
    sys.path.insert(0, "/opt/trn_rl_repo")

import numpy as np

import concourse.bacc as bacc
import concourse.mybir as mybir
from concourse.tile import TileContext

N_CORES = 8
NS = 2          # samples per core
C, H, W = 128, 128, 128
F32 = mybir.dt.float32

# (channel_start, n_channels, block_h, block_w); block_h None => identity
GROUPS = [
    (0, 32, 1, 1),
    (32, 32, 2, 2),
    (64, 32, 4, 4),
    (96, 16, 2, 4),
    (112, 16, None, None),
]


def _emit_group(nc, pools, act, out, n, c0, gc, bh, bw):
    kc = 128 // gc          # H-chunks per channel
    r = H // kc             # rows per chunk (== gc here)
    fs = r * W              # free elements per partition

    px, ps1, ps2, pw1, pw2 = pools
    x = px.tile([128, fs], F32, tag="x")

    # group block is contiguous in HBM: [gc, H, W] from channel c0
    src = act[n, c0 : c0 + gc].flatten().rearrange("(p f) -> p f", p=128)
    dst = out[n, c0 : c0 + gc].flatten().rearrange("(p f) -> p f", p=128)

    nc.sync.dma_start(x[:], src)

    if bh is not None and bh * bw > 1:
        nh = r // bh
        nw = W // bw

        # H reduction: pairwise row adds until one row per h-block
        cur, rows = x, r
        while rows > nh:
            nxt = (ps1 if rows == r else ps2).tile(
                [128, (rows // 2) * W], F32, tag="s1" if rows == r else "s2"
            )
            v = cur[:].rearrange("p (b t w) -> p b t w", t=2, w=W)
            nc.vector.tensor_add(
                nxt[:].rearrange("p (b w) -> p b w", w=W),
                v[:, :, 0, :],
                v[:, :, 1, :],
            )
            cur, rows = nxt, rows // 2

        # W reduction: pairwise column adds until one value per block
        cols = W
        while cols > nw:
            nxt = (pw1 if cols == W else pw2).tile(
                [128, nh * (cols // 2)], F32, tag="w1" if cols == W else "w2"
            )
            v = cur[:].rearrange("p (b c t) -> p b c t", b=nh, t=2)
            nc.vector.tensor_add(
                nxt[:].rearrange("p (b c) -> p b c", b=nh),
                v[:, :, :, 0],
                v[:, :, :, 1],
            )
            cur, cols = nxt, cols // 2

        # Gate: out = (block_sum >= 0) * x, one op per row offset in block
        msum = cur[:].rearrange("p (b wb) -> p b wb", wb=nw)
        mbc = msum.unsqueeze(3).broadcast_to([128, nh, nw, bw])
        for hi in range(bh):
            xv = (
                x[:]
                .rearrange("p (b t w) -> p b t w", t=bh, w=W)[:, :, hi, :]
                .rearrange("p b (wb wi) -> p b wb wi", wi=bw)
            )
            nc.vector.scalar_tensor_tensor(
                xv, mbc, 0.0, xv, mybir.AluOpType.is_ge, mybir.AluOpType.mult
            )
    elif bh == 1:
        # ReLU channels
        nc.vector.tensor_scalar_max(x[:], x[:], 0.0)
    # else identity: store as-is

    nc.scalar.dma_start(dst, x[:])


def build_bass():
    nc = bacc.Bacc(
        "TRN2", target_bir_lowering=False, debug=False, num_devices=N_CORES
    )
    act = nc.dram_tensor("activation", [NS, C, H, W], F32, kind="ExternalInput")
    out = nc.dram_tensor("out", [NS, C, H, W], F32, kind="ExternalOutput")
    with TileContext(nc) as tc:
        with (
            tc.tile_pool(name="x", bufs=4) as px,
            tc.tile_pool(name="s1", bufs=2) as ps1,
            tc.tile_pool(name="s2", bufs=2) as ps2,
            tc.tile_pool(name="w1", bufs=2) as pw1,
            tc.tile_pool(name="w2", bufs=2) as pw2,
        ):
            pools = (px, ps1, ps2, pw1, pw2)
            for n in range(NS):
                for c0, gc, bh, bw in GROUPS:
                    _emit_group(nc, pools, act, out, n, c0, gc, bh, bw)
    nc.compile()
    return nc


_NC = None


def _get_nc():
    global _NC
    if _NC is None:
        _NC = build_bass()
    return _NC


def run(activation, trace=False, **spmd_kwargs):
    from concourse.bass_utils import run_bass_kernel_spmd

    activation = np.ascontiguousarray(np.asarray(activation), dtype=np.float32)
    assert activation.shape == (N_CORES * NS, C, H, W), activation.shape
    nc = _get_nc()
    in_maps = [
        {"activation": activation[i * NS : (i + 1) * NS]} for i in range(N_CORES)
    ]
    res = run_bass_kernel_spmd(
        nc, in_maps, core_ids=list(range(N_CORES)), trace=trace, **spmd_kwargs
    )
    full = np.concatenate([r["out"] for r in res.results], axis=0)
    return full, res


def kernel(activation):
    return run(activation)[0]


if __name__ == "__main__":
    rng = np.random.default_rng(0)
    a = rng.standard_normal((16, 128, 128, 128), dtype=np.float32)
    y = kernel(a)
    print("ran:", y.shape, y.dtype)
